# revision 14
# baseline (speedup 1.0000x reference)
"""GQA attention layer (B=2, L=2048, D=4096, H=32, KH=8, HD=128) on 8 TRN2 cores.

v3: projections emit q/k directly in [hd, tok] layout (weights stationary,
x moving) so no DMA transposes; per-head RMS norm via gpsimd
partition_all_reduce (replicated sums -> full-tile rsqrt path, no
broadcasts); rope applied from packed [hd, tok] tables with the softmax
scale and norm weights folded in; softmax denominators via gpsimd
partition_all_reduce (no ones/broadcast matmuls on PE); attention exp done
in 2-bank PSUM stripes (fewer Act instructions); o-proj split into an
h0-2 pass and an h3 pass so the last AllToAll is fully hidden.

Sharding: tensor-parallel over KV heads (1 kv head + 4 q heads per core),
AllToAll to redistribute attention outputs token-wise, each core runs the
output projection for its 512-token slice. Host concatenates slices.
"""
import numpy as np
import ml_dtypes

import concourse.bass as bass
import concourse.mybir as mybir
import concourse.tile as tile
import concourse.bass_isa as bass_isa
from concourse import bacc
from concourse.bass_utils import run_bass_kernel_spmd

F32 = mybir.dt.float32
BF = mybir.dt.bfloat16
AF = mybir.ActivationFunctionType
MUL = mybir.AluOpType.mult
ADD = mybir.AluOpType.add
RADD = bass_isa.ReduceOp.add

B, L, D = 2, 2048, 4096
H, KH, HD = 32, 8, 128
T = B * L              # 4096 tokens
NC_ = 8                # cores
QH = H // NC_          # 4 q heads per core
NT = T // 128          # 32 token tiles
NB = T // 512          # 8 token blocks
QB = 512               # q block
EPS = 1e-5
ROPE_BASE = 1000000.0
NPBF = ml_dtypes.bfloat16

_CACHE = {}


def _build():
    nc = bacc.Bacc("TRN2", target_bir_lowering=False, debug=False, num_devices=NC_)

    xq = nc.dram_tensor("xq", [128, NB, 32, 512], BF, kind="ExternalInput").ap()
    wproj = nc.dram_tensor("wproj", [6, 128, 32, 128], BF, kind="ExternalInput").ap()
    ropeA = nc.dram_tensor("ropeA", [128, 2, L], BF, kind="ExternalInput").ap()
    ropeB = nc.dram_tensor("ropeB", [128, 2, L], BF, kind="ExternalInput").ap()
    patd = nc.dram_tensor("patd", [128, 128], BF, kind="ExternalInput").ap()
    wop = nc.dram_tensor("wop", [8, QH, 128, 8, 512], BF, kind="ExternalInput").ap()
    out = nc.dram_tensor("out", [T // NC_, D], F32, kind="ExternalOutput").ap()

    with tile.TileContext(nc) as tc:
        with (
            tc.tile_pool(name="const", bufs=1) as cp,
            tc.tile_pool(name="dram", bufs=1, space="DRAM") as dramp,
            tc.tile_pool(name="kv", bufs=1) as kvp,
        ):
            pat_sb = cp.tile([128, 128], BF)
            nc.sync.dma_start(pat_sb[:], patd)

            kT_sb = kvp.tile([128, T], BF)            # [hd, tok]
            qT_sb = [kvp.tile([128, T], BF, name=f"qT{h}") for h in range(QH)]
            v_sb = kvp.tile([128, NT, 128], BF)       # [tok%128, tile, hd]

            a2a_in = [dramp.tile([NC_, 128, QB], BF, name=f"a2ain{h}")
                      for h in range(QH)]
            a2a_out = [dramp.tile([NC_, 128, QB], BF, name=f"a2aout{h}")
                       for h in range(QH)]

            # ---------------- phase 1: projections + norm + rope ----------
            with (
                tc.tile_pool(name="wts", bufs=1) as wp,
                tc.tile_pool(name="px", bufs=2) as px,
                tc.tile_pool(name="p1", bufs=2) as p1,
                tc.tile_pool(name="ps1", bufs=3, space="PSUM") as ps1,
            ):
                # chunk order: k first, then v, then q heads
                CHUNKS = [4, 5, 0, 1, 2, 3]
                w_sb = wp.tile([128, 6, 32, 128], BF)
                nc.sync.dma_start(w_sb[:, 4, :, :], wproj[4, :, :, :])
                for c_ in [5, 0, 1, 2, 3]:
                    nc.scalar.dma_start(w_sb[:, c_, :, :], wproj[c_, :, :, :])
                rA = wp.tile([128, 2, L], BF)
                nc.scalar.dma_start(rA[:], ropeA)
                rB = wp.tile([128, 2, L], BF)
                nc.scalar.dma_start(rB[:], ropeB)

                norm_pend = []

                def flush_norm():
                    while norm_pend:
                        (S_, rr_, dst_) = norm_pend.pop(0)
                        var = p1.tile([128, 512], F32, tag="var", name="var")
                        nc.vector.tensor_scalar(var[:], S_[:], 1.0 / HD, EPS,
                                                MUL, ADD)
                        rms = p1.tile([128, 512], F32, tag="rms", name="rms")
                        nc.scalar.activation(rms[:], var[:], AF.Sqrt)
                        inv = p1.tile([128, 512], BF, tag="inv", name="inv")
                        with nc.allow_low_precision(reason="bf16 rms recip"):
                            nc.vector.reciprocal(inv[:], rms[:])
                        nc.vector.tensor_tensor(dst_, rr_[:], inv[:], MUL)

                for blk in range(NB):
                    xt = px.tile([128, 32, 512], BF, tag="xt", name="xt")
                    for o8 in range(4):
                        nc.sync.dma_start(xt[:, 8 * o8:8 * (o8 + 1), :],
                                          xq[:, blk, 8 * o8:8 * (o8 + 1), :])
                    pos0 = 512 * (blk % 4)
                    blk_chunks = CHUNKS if blk < NB - 1 else [0, 4, 1, 2, 3, 5]
                    for c_ in blk_chunks:
                        ps = ps1.tile([128, 512], F32, tag="ps", name="ps")
                        for o in range(32):
                            nc.tensor.matmul(ps[:], w_sb[:, c_, o, :], xt[:, o, :],
                                             start=(o == 0), stop=(o == 31))
                        if c_ == 5:
                            # v: copy out and transpose to [tok, hd]
                            vst = p1.tile([128, 512], BF, tag="vst", name="vst")
                            nc.scalar.copy(vst[:], ps[:])
                            for t4 in range(4):
                                nc.scalar.dma_start_transpose(
                                    v_sb[:, 4 * blk + t4, :],
                                    vst[:, 128 * t4:128 * (t4 + 1)])
                            continue
                        kind = 1 if c_ == 4 else 0
                        qc = p1.tile([128, 512], BF, tag="qc", name="qc")
                        nc.scalar.copy(qc[:], ps[:])
                        flush_norm()
                        # rope pieces (independent of the RMS chain)
                        t1 = p1.tile([128, 512], BF, tag="t1", name="t1")
                        nc.vector.tensor_tensor(
                            t1[:], qc[:], rA[:, kind, pos0:pos0 + 512], MUL)
                        qs = p1.tile([128, 512], BF, tag="qs", name="qs")
                        nc.vector.tensor_copy(out=qs[0:64, :], in_=qc[64:128, :])
                        nc.vector.tensor_copy(out=qs[64:128, :], in_=qc[0:64, :])
                        t2 = p1.tile([128, 512], BF, tag="t2", name="t2")
                        nc.vector.tensor_tensor(
                            t2[:], qs[:], rB[:, kind, pos0:pos0 + 512], MUL)
                        rr = p1.tile([128, 512], BF, tag="rr", name="rr")
                        nc.vector.tensor_tensor(rr[:], t1[:], t2[:], ADD)
                        # RMS: sumsq replicated across partitions via gpsimd
                        sq = p1.tile([128, 512], BF, tag="sq", name="sq")
                        nc.vector.tensor_tensor(sq[:], qc[:], qc[:], MUL)
                        S = p1.tile([128, 512], BF, tag="S", name="S")
                        nc.gpsimd.partition_all_reduce(S[:], sq[:], channels=128,
                                                       reduce_op=RADD)
                        if kind == 1:
                            dst = kT_sb[:, 512 * blk:512 * (blk + 1)]
                        else:
                            dst = qT_sb[c_][:, 512 * blk:512 * (blk + 1)]
                        norm_pend.append((S, rr, dst))
                flush_norm()

            # ---------------- phase 2: attention + per-head AllToAll --------
            atp_cm = tc.tile_pool(name="at", bufs=1)
            atp = atp_cm.__enter__()
            at_sb = [atp.tile([128, NC_, QB], BF, name=f"at{h}") for h in range(QH)]
            ob_sb = atp.tile([128, 4, 8, 512], F32, name="ob")  # [tok%128,tt,oc,n]
            with (
                tc.tile_pool(name="p2", bufs=4) as p2,
                tc.tile_pool(name="ps2", bufs=2, space="PSUM") as ps2,
                tc.tile_pool(name="pso", bufs=2, space="PSUM") as pso,
                tc.tile_pool(name="p4w", bufs=3) as p4w,
                tc.tile_pool(name="p4ps", bufs=2, space="PSUM") as p4ps,
            ):
                # o-proj work interleaved into attention's Act-bound bubbles:
                # groups of 8 matmuls (one [oc, tt] output tile for one head)
                p4q = []
                p4wt = {}

                def p4_prefetch():
                    seen = []
                    for (h_, oc_, tt_) in p4q:
                        if (h_, oc_) not in seen:
                            seen.append((h_, oc_))
                        if len(seen) == 2:
                            break
                    for key in seen:
                        if key not in p4wt:
                            wt = p4w.tile([128, 8, 512], BF, tag="wt", name="wt")
                            nc.sync.dma_start(wt[:], wop[key[1], key[0], :, :, :])
                            p4wt[key] = wt

                def p4_drain(n):
                    for _ in range(n):
                        if not p4q:
                            return
                        p4_prefetch()
                        (h_, oc_, tt_) = p4q.pop(0)
                        wt = p4wt[(h_, oc_)]
                        po2 = p4ps.tile([128, 512], F32, tag="po2", name="po2")
                        for s in range(8):
                            nc.tensor.matmul(
                                po2[:],
                                at_sb[h_][:, s, 128 * tt_:128 * (tt_ + 1)],
                                wt[:, s, :], start=(s == 0), stop=(s == 7))
                        dst = ob_sb[:, tt_, oc_, :]
                        if h_ == 0:
                            nc.vector.tensor_copy(out=dst, in_=po2[:])
                        else:
                            nc.vector.tensor_tensor(dst, dst, po2[:], ADD)
                        if h_ == 3:
                            nc.scalar.dma_start(
                                out[128 * tt_:128 * (tt_ + 1),
                                    512 * oc_:512 * (oc_ + 1)], dst)
                        if tt_ == 3:
                            del p4wt[(h_, oc_)]

                def emit_out(ent):
                    (pT, cols, pso_o, nkt) = ent
                    for u, (kt, col0, ktile) in enumerate(cols):
                        nc.tensor.matmul(pso_o[:, col0:QB], v_sb[:, ktile, :],
                                         pT[:, u, col0:QB],
                                         start=(kt == 0), stop=(kt == nkt - 1))

                def emit_tail(ent):
                    (h, j, acc, pso_o) = ent
                    pso_c = p2.tile([128, QB], BF, tag="pso_c", name="pso_c",
                                    bufs=4)
                    nc.scalar.copy(pso_c[:], pso_o[:])   # frees the PSUM bank
                    S2 = p2.tile([128, QB], F32, tag="S2", name="S2", bufs=3)
                    nc.gpsimd.partition_all_reduce(S2[:], acc[:], channels=128,
                                                   reduce_op=RADD)
                    R = p2.tile([128, QB], F32, tag="R", name="R", bufs=3)
                    nc.vector.reciprocal_approx_fast(R[:], S2[:])
                    attn = p2.tile([128, QB], BF, tag="attn", name="attn", bufs=7)
                    nc.vector.tensor_tensor(attn[:], pso_c[:], R[:], MUL)
                    nc.sync.dma_start(a2a_in[h][j, :, :], attn[:])

                pend_tail = None
                pend = []
                for h in range(QH):
                    if h >= 2:
                        p4q.extend((h - 2, oc_, tt_)
                                   for oc_ in range(8) for tt_ in range(4))
                    for b in range(B):
                        for qb in range(4):
                            nkt = 4 * qb + 4
                            j = 4 * b + qb
                            q0 = (b * 16 + 4 * qb) * 128
                            acc = p2.tile([128, QB], BF, tag="acc", name="acc",
                                          bufs=4)
                            pso_o = pso.tile([128, QB], F32, tag="o", name="pso_o")
                            for st in range(nkt // 2):
                                pss = ps2.tile([128, 2, QB], F32, tag="pss",
                                               name="pss")
                                cols = []
                                for u in range(2):
                                    kt = 2 * st + u
                                    t = kt - 4 * qb
                                    col0 = 128 * t if t > 0 else 0
                                    ktile = b * 16 + kt
                                    nc.tensor.matmul(
                                        pss[:, u, col0:QB],
                                        kT_sb[:, 128 * ktile:128 * (ktile + 1)],
                                        qT_sb[h][:, q0 + col0:q0 + QB],
                                        start=True, stop=True)
                                    cols.append((kt, col0, ktile))
                                while len(pend) >= 3:
                                    emit_out(pend.pop(0))
                                if pend_tail is not None and st == 1:
                                    # finish the previous block's attn@V, then
                                    # normalize + ship it
                                    while pend and pend[0][2] is pend_tail[3]:
                                        emit_out(pend.pop(0))
                                    emit_tail(pend_tail)
                                    pend_tail = None
                                pT = p2.tile([128, 2, QB], BF, tag="pT",
                                             name="pT", bufs=5)
                                diag = cols[1][1] > cols[0][1]
                                if diag:
                                    for u, (kt, col0, ktile) in enumerate(cols):
                                        nc.scalar.activation(
                                            pT[:, u, col0:QB],
                                            pss[:, u, col0:QB], AF.Exp)
                                else:
                                    nc.scalar.activation(pT[:], pss[:], AF.Exp)
                                for u, (kt, col0, ktile) in enumerate(cols):
                                    t = kt - 4 * qb
                                    if t >= 0:
                                        nc.vector.tensor_tensor(
                                            pT[:, u, col0:col0 + 128],
                                            pT[:, u, col0:col0 + 128],
                                            pat_sb[:], MUL)
                                with nc.allow_low_precision(
                                        reason="bf16 softmax denominator acc"):
                                    for u, (kt, col0, ktile) in enumerate(cols):
                                        if kt == 0:
                                            nc.vector.tensor_copy(
                                                out=acc[:], in_=pT[:, 0, :])
                                        else:
                                            nc.vector.tensor_tensor(
                                                acc[:, col0:QB], acc[:, col0:QB],
                                                pT[:, u, col0:QB], ADD)
                                pend.append((pT, cols, pso_o, nkt))
                            if pend_tail is not None:
                                while pend and pend[0][2] is pend_tail[3]:
                                    emit_out(pend.pop(0))
                                emit_tail(pend_tail)
                            pend_tail = (h, j, acc, pso_o)
                            p4_drain(4)
                    while pend:
                        emit_out(pend.pop(0))
                    emit_tail(pend_tail)
                    pend_tail = None
                    nc.gpsimd.collective_compute(
                        "AllToAll", mybir.AluOpType.bypass,
                        replica_groups=[list(range(NC_))],
                        ins=[a2a_in[h].opt()], outs=[a2a_out[h].opt()])
                    if h > 0:
                        # gpsimd park only delays the (buffered) denominator
                        # chain of the next head, never the PE path
                        nc.gpsimd.dma_start(
                            at_sb[h - 1][:],
                            a2a_out[h - 1].rearrange("s p t -> p s t"))
                # gpsimd is idle after the last collective; its queue ordering
                # gives exactly the a2a(3)-completion dependency
                nc.gpsimd.dma_start(at_sb[3][:],
                                    a2a_out[3].rearrange("s p t -> p s t"))
                # tail: remaining o-proj (h2 groups cover the last AllToAll)
                p4q.extend((h_, oc_, tt_) for h_ in (2, 3)
                           for oc_ in range(8) for tt_ in range(4))
                p4_drain(len(p4q))
            atp_cm.__exit__(None, None, None)

    nc.compile()
    return nc


def _prep(inputs):
    x = np.asarray(inputs["x"], np.float32)
    wq = np.asarray(inputs["wq"], np.float32)
    wk = np.asarray(inputs["wk"], np.float32)
    wv = np.asarray(inputs["wv"], np.float32)
    wo = np.asarray(inputs["wo"], np.float32)
    qw = np.asarray(inputs["q_norm_w"], np.float32)
    kw = np.asarray(inputs["k_norm_w"], np.float32)

    xf = np.ascontiguousarray(x.reshape(T, D))
    # xq[p, blk, o, t] = xf[512*blk + t, 128*o + p]
    xqp = np.ascontiguousarray(
        xf.reshape(NB, 512, 32, 128).transpose(3, 0, 2, 1).astype(NPBF))

    half = HD // 2
    inv_freq = 1.0 / (ROPE_BASE ** (np.arange(half, dtype=np.float32) / half))
    pos = np.arange(L, dtype=np.float32)
    ang = pos[:, None] * inv_freq[None, :]          # [L, 64]
    cos = np.cos(ang).astype(np.float32)
    sin = np.sin(ang).astype(np.float32)
    scale = np.float32(HD ** -0.5)

    # A[p<64] = cos*w[p]*s ; A[p>=64] = cos*w[p]*s (same freq block)
    # B[p<64] = -sin*w[64+p]*s ; B[p>=64] = sin*w[p-64]*s
    def rope_tabs(w, s):
        A = np.empty((128, L), np.float32)
        Bt = np.empty((128, L), np.float32)
        A[0:64] = (cos * w[None, 0:64] * s).T
        A[64:128] = (cos * w[None, 64:128] * s).T
        Bt[0:64] = (-sin * w[None, 64:128] * s).T
        Bt[64:128] = (sin * w[None, 0:64] * s).T
        return A, Bt

    Aq, Bq = rope_tabs(qw, scale)
    Ak, Bk = rope_tabs(kw, np.float32(1.0))
    ropeA = np.ascontiguousarray(
        np.stack([Aq, Ak], axis=1).astype(NPBF))     # [128, 2, L]
    ropeB = np.ascontiguousarray(
        np.stack([Bq, Bk], axis=1).astype(NPBF))

    kk = np.arange(128)[:, None]
    jj = np.arange(128)[None, :]
    patd = (kk <= jj).astype(NPBF)

    # wop[oc, h, p, s, n] = wo[(4s+h)*128 + p, 512*oc + n]
    wop = np.ascontiguousarray(
        wo.reshape(8, QH, 128, 8, 512).transpose(3, 1, 2, 0, 4).astype(NPBF))

    in_maps = []
    for c in range(NC_):
        wq_c = wq[:, 512 * c:512 * (c + 1)]
        wk_c = wk[:, HD * c:HD * (c + 1)]
        wv_c = wv[:, HD * c:HD * (c + 1)]
        wcat = np.concatenate([wq_c, wk_c, wv_c], axis=1)    # [4096, 768]
        # wproj[c_, p, o, m] = wcat[128*o + p, 128*c_ + m]
        wproj = np.ascontiguousarray(
            wcat.reshape(32, 128, 6, 128).transpose(2, 1, 0, 3).astype(NPBF))
        in_maps.append({
            "xq": xqp,
            "wproj": wproj,
            "ropeA": ropeA,
            "ropeB": ropeB,
            "patd": patd,
            "wop": wop,
        })
    return in_maps


def kernel(**inputs) -> np.ndarray:
    if "nc" not in _CACHE:
        _CACHE["nc"] = _build()
    nc = _CACHE["nc"]
    in_maps = _prep(inputs)
    res = run_bass_kernel_spmd(nc, in_maps, list(range(NC_)))
    chunks = [res.results[c]["out"] for c in range(NC_)]
    return np.concatenate(chunks, axis=0).reshape(B, L, D)
